# revision 18
# baseline (speedup 1.0000x reference)
"""Gridnet Trainium2 kernel — self-contained.

Blockwise-normalized 27-neighbor gridnet step (8 inner iterations,
block_size 8) for x:[4,128,128,128] f32 with per-cell weights
w:[27,128,128,128], bias, residual_scale.

Strategy
--------
Shard the M axis across the 8 NeuronCores (16 M-cells = 2 blocks of 8 per
core). Blocks carry a frozen 1-cell halo from the initial activations, so
cores never communicate.

Per core: 512 spatial blocks x 4 batches. On-chip layout: 4 tiles of 128
blocks (partition = block). Free dim per tile: padded cells x batch stored
as [bh(2), i(10), j(10), k(10), bl(2)] (batch b = 2*bh+bl) so any stencil
window merges (k,bl) into one contiguous stride-1 dim of 16 -> all stencil
passes hit the DVE bf16 2x mode with 3-dim APs:
    window(offset di,dj,dk; half bh) = [(200,8),(20,8),(1,16)].

Math: normalization is affine per block, so the 27-offset weighted sum S
runs on RAW activations and is corrected afterward:
    acc = istd*(S - mu*sw + std*bias),  sw = sum_o w_o (precomputed).
The 27 products run on the Vector engine in bf16 (2x mode); the TensorE
accumulates them into PSUM in f32 via identity matmuls (no DVE adds); the
-mu*sw (+std*bias) corrections are built on ScalarE and folded into the
same PSUM accumulation; silu reads PSUM directly on ScalarE. Stats
(sum/sumsq) run on ScalarE via activation accum_out.

Weights are streamed from HBM every iteration (too big for SBUF),
replicated x2 over the inner batch pair to match the (k,bl)-merged layout
(x2 rather than x4 keeps the stream at ~28 MB/iter/core, under the
~358 GB/s per-core DMA ceiling).
"""
import numpy as np
import ml_dtypes

BS = 8
EPS = 1e-5
N_CORES = 8
BF = ml_dtypes.bfloat16

_CACHE = {}


# ---------------------------------------------------------------- host prep

def _blk(v):
    """[..., 16,128,128] -> [..., 4(tile),128(p),512(cell ii,jj,kk)]."""
    L = v.shape[:-3]
    vr = v.reshape(*L, 2, 8, 2, 8, 8, 16, 8)   # g, ii, nh, nl, jj, kb, kk
    n = len(L)
    perm = tuple(range(n)) + tuple(n + i for i in (0, 2, 3, 5, 1, 4, 6))
    return vr.transpose(perm).reshape(*L, 4, 128, 512)


def _prep_inputs(weight, bias, residual_scale, x):
    xp = np.pad(x, ((0, 0), (1, 1), (1, 1), (1, 1)))
    swv = np.lib.stride_tricks.sliding_window_view(xp, (10, 10, 10), axis=(1, 2, 3))
    blocks = swv[:, ::8, ::8, ::8]            # [4b,16,16,16,10,10,10]
    wsum = weight.sum(axis=0)                 # [128,128,128] f32

    in_maps = []
    for c in range(N_CORES):
        g2 = blocks[:, 2 * c:2 * c + 2]       # [4b,2g,16n,16kb,10,10,10]
        arr = g2.reshape(2, 2, 2, 2, 8, 16, 10, 10, 10)  # bh,bl,g,nh,nl,kb,ci,cj,ck
        arr = arr.transpose(2, 3, 4, 5, 0, 6, 7, 8, 1)   # g,nh,nl,kb,bh,ci,cj,ck,bl
        a_host = np.ascontiguousarray(arr).reshape(4, 128, 4000).astype(BF)

        wb = _blk(weight[:, 16 * c:16 * c + 16])      # [27,4,128,512]
        w2 = np.repeat(wb.transpose(1, 0, 2, 3).astype(BF)[..., None], 2, axis=-1)
        w2 = w2.reshape(4, 9, 3, 128, 1024).transpose(0, 1, 3, 2, 4)
        w2 = np.ascontiguousarray(w2).reshape(4, 9, 128, 3072)

        sw_b = np.ascontiguousarray(_blk(wsum[16 * c:16 * c + 16]))          # f32
        bias_b = np.ascontiguousarray(_blk(bias[16 * c:16 * c + 16]).astype(BF))
        rs_b = _blk(residual_scale[16 * c:16 * c + 16]).astype(BF)  # [4,128,512]
        rs1 = np.repeat(rs_b[..., None], 2, axis=-1).reshape(4, 128, 1024)
        rs2 = np.ascontiguousarray(np.concatenate([rs1, rs1], axis=-1))  # [bh dup]
        in_maps.append({'a_in': a_host, 'w2_in': w2, 'sw_in': sw_b,
                        'bias_in': bias_b, 'rs2_in': rs2,
                        'ident_in': np.eye(128, dtype=BF)})
    return in_maps


def _unpack_outputs(results):
    y = np.empty((4, 128, 128, 128), np.float32)
    for c in range(N_CORES):
        out = results[c]['out_f']             # [4,128,2048] f32
        arr = out.reshape(2, 2, 8, 16, 2, 8, 8, 8, 2)  # g,nh,nl,kb,bh,ii,jj,kk,bl
        yt = arr.transpose(4, 8, 0, 5, 1, 2, 6, 3, 7)  # bh,bl,g,ii,nh,nl,jj,kb,kk
        y[:, 16 * c:16 * c + 16] = yt.reshape(4, 16, 128, 128)
    return y


# ---------------------------------------------------------------- device code

def _build(iters, zero_bias, unit_rs):
    import concourse.bass as bass
    import concourse.tile as tile
    from concourse import bacc, mybir

    BF16 = mybir.dt.bfloat16
    F32 = mybir.dt.float32
    ALU = mybir.AluOpType
    ACTF = mybir.ActivationFunctionType

    def wap(t, offset, dims):
        ap = t.ap().copy()
        ap.ap = mybir.VecI64Pair([list(t.ap().ap[0])] + [list(d) for d in dims])
        ap.offset = offset
        return ap

    nc = bacc.Bacc('TRN2', target_bir_lowering=False, debug=False)
    a_in = nc.dram_tensor('a_in', [4, 128, 4000], BF16, kind='ExternalInput').ap()
    w2_in = nc.dram_tensor('w2_in', [4, 9, 128, 3072], BF16, kind='ExternalInput').ap()
    sw_in = nc.dram_tensor('sw_in', [4, 128, 512], F32, kind='ExternalInput').ap()
    bias_in = nc.dram_tensor('bias_in', [4, 128, 512], BF16, kind='ExternalInput').ap()
    rs2_in = nc.dram_tensor('rs2_in', [4, 128, 2048], BF16, kind='ExternalInput').ap()
    ident_in = nc.dram_tensor('ident_in', [128, 128], BF16, kind='ExternalInput').ap()
    out_f = nc.dram_tensor('out_f', [4, 128, 2048], F32, kind='ExternalOutput').ap()

    # A free-dim layout: bh*2000 + ci*200 + cj*20 + ck*2 + bl
    WDIMS = [[200, 8], [20, 8], [1, 16]]   # 8x8x(8k x 2bl) window, per bh
    # POST (tmp/psum/sil/msw/fo) layout: bh*1024 + ii*128 + jj*16 + kk*2 + bl
    PDIMS = [[128, 8], [16, 8], [1, 16]]
    BDIMS = [[128, 8], [16, 8], [2, 8]]    # per-(bh,bl) slice of POST

    with tile.TileContext(nc) as tc:
        with (
            tc.tile_pool(name='persist', bufs=1) as ppool,
            tc.tile_pool(name='wstream', bufs=9) as wpool,
            tc.tile_pool(name='work', bufs=2) as kpool,
            tc.tile_pool(name='tmppool', bufs=8) as tpool,
            tc.tile_pool(name='small', bufs=2) as spool,
            tc.tile_pool(name='psum', bufs=2, space='PSUM') as pspool,
        ):
            A = []
            for t in range(4):
                at = ppool.tile([128, 4000], BF16, tag=f'A{t}')
                for s4 in range(4):
                    nc.sync.dma_start(at[:, s4 * 1000:(s4 + 1) * 1000],
                                      a_in[t][:, s4 * 1000:(s4 + 1) * 1000])
                A.append(at)
            ident = ppool.tile([128, 128], BF16, tag='ident')
            nc.sync.dma_start(ident[:], ident_in)
            eps_t = ppool.tile([128, 1], F32, tag='eps')
            nc.vector.memset(eps_t[:], EPS)
            swt, biast, rst = [], [], []
            for t in range(4):
                s = ppool.tile([128, 512], F32, tag=f'sw{t}')
                nc.sync.dma_start(s[:], sw_in[t])
                swt.append(s)
                if not zero_bias:
                    bb = ppool.tile([128, 512], BF16, tag=f'bias{t}')
                    nc.sync.dma_start(bb[:], bias_in[t])
                    biast.append(bb)
                if not unit_rs:
                    r = ppool.tile([128, 2048], BF16, tag=f'rs{t}')
                    nc.sync.dma_start(r[:], rs2_in[t])
                    rst.append(r)

            for it in range(iters):
                last = it == iters - 1
                for t in range(4):
                    At = A[t]
                    # ---- stats on ScalarE: per-batch sum & sumsq over 1000 cells
                    s_t = spool.tile([128, 4], F32, tag='s')
                    q_t = spool.tile([128, 4], F32, tag='q')
                    junk = spool.tile([128, 1000], BF16, tag='junk')
                    junkf = spool.tile([128, 1000], F32, tag='junkf')
                    for b in range(4):
                        bh, bl = b // 2, b % 2
                        cells = wap(At.tensor, bh * 2000 + bl,
                                    [[200, 10], [20, 10], [2, 10]])
                        nc.scalar.activation(junk[:], cells, ACTF.Copy,
                                             accum_out=s_t[:, b:b + 1])
                        nc.scalar.activation(junkf[:], cells, ACTF.Square,
                                             accum_out=q_t[:, b:b + 1])
                    # ---- stat scalars: negmu, std, istd  [128,4]
                    negmu = spool.tile([128, 4], F32, tag='negmu')
                    nc.scalar.mul(negmu[:], s_t[:], -1.0 / 1000.0)
                    m2 = spool.tile([128, 4], F32, tag='m2')
                    nc.scalar.mul(m2[:], q_t[:], 1.0 / 1000.0)
                    musq = spool.tile([128, 4], F32, tag='musq')
                    nc.scalar.square(musq[:], negmu[:])
                    var = spool.tile([128, 4], F32, tag='var')
                    nc.vector.tensor_tensor(var[:], m2[:], musq[:], ALU.subtract)
                    std = spool.tile([128, 4], F32, tag='std')
                    nc.scalar.activation(std[:], var[:], ACTF.Sqrt, bias=eps_t[:])
                    istd = spool.tile([128, 4], F32, tag='istd')
                    nc.vector.reciprocal(istd[:], std[:])

                    # ---- stencil: DVE bf16 products; TensorE identity-matmuls
                    # accumulate the 27 offsets into PSUM in f32
                    ps = pspool.tile([128, 2048], F32, tag='ps')
                    for c9 in range(9):
                        wt = wpool.tile([128, 3072], BF16, tag='w')
                        nc.sync.dma_start(wt[:], w2_in[t, c9])
                        for ol in range(3):
                            o = c9 * 3 + ol
                            di, dj, dk = o // 9, (o // 3) % 3, o % 3
                            tmp = tpool.tile([128, 2048], BF16, tag='tmp')
                            for bh in range(2):
                                win = wap(At.tensor,
                                          bh * 2000 + di * 200 + dj * 20 + dk * 2,
                                          WDIMS)
                                wsl = wap(wt.tensor, ol * 1024, PDIMS)
                                to = wap(tmp.tensor, bh * 1024, PDIMS)
                                nc.vector.tensor_tensor(to, wsl, win, ALU.mult)
                            for q in range(4):
                                nc.tensor.matmul(
                                    ps[:, q * 512:(q + 1) * 512], ident[:],
                                    tmp[:, q * 512:(q + 1) * 512],
                                    start=(o == 0), stop=False)
                    # fold the normalization correction into PSUM via ScalarE:
                    # msw[., b] = -mu_b * sw  (+ std_b * bias), as extra matmuls
                    msw = kpool.tile([128, 2048], BF16, tag='msw')
                    for b in range(4):
                        bh, bl = b // 2, b % 2
                        mswb = wap(msw.tensor, bh * 1024 + bl, BDIMS)
                        nc.scalar.mul(mswb, swt[t][:], negmu[:, b:b + 1])
                    for q in range(4):
                        nc.tensor.matmul(
                            ps[:, q * 512:(q + 1) * 512], ident[:],
                            msw[:, q * 512:(q + 1) * 512],
                            start=False, stop=zero_bias)
                    if not zero_bias:
                        bsw = kpool.tile([128, 2048], BF16, tag='bsw')
                        for b in range(4):
                            bh, bl = b // 2, b % 2
                            bswb = wap(bsw.tensor, bh * 1024 + bl, BDIMS)
                            nc.scalar.mul(bswb, biast[t][:], std[:, b:b + 1])
                        for q in range(4):
                            nc.tensor.matmul(
                                ps[:, q * 512:(q + 1) * 512], ident[:],
                                bsw[:, q * 512:(q + 1) * 512],
                                start=False, stop=True)

                    # ---- silu straight off PSUM (ScalarE sits next to PSUM)
                    sil = kpool.tile([128, 2048], BF16, tag='sil')
                    for b in range(4):
                        bh, bl = b // 2, b % 2
                        psb = wap(ps.tensor, bh * 1024 + bl, BDIMS)
                        silb = wap(sil.tensor, bh * 1024 + bl, BDIMS)
                        nc.scalar.activation(silb, psb, ACTF.Silu,
                                             scale=istd[:, b:b + 1])
                    # ---- residual update
                    if unit_rs:
                        delta = sil
                    else:
                        delta = kpool.tile([128, 2048], BF16, tag='delta')
                        nc.vector.tensor_tensor(delta[:], rst[t][:], sil[:], ALU.mult)
                    fo = None
                    if last:
                        fo = kpool.tile([128, 2048], F32, tag='fo')
                    for bh in range(2):
                        awin = wap(At.tensor, bh * 2000 + 222, WDIMS)
                        dwin = wap(delta.tensor, bh * 1024, PDIMS)
                        if not last:
                            nc.vector.tensor_tensor(awin, awin, dwin, ALU.add)
                        else:
                            fwin = wap(fo.tensor, bh * 1024, PDIMS)
                            nc.vector.tensor_tensor(fwin, awin, dwin, ALU.add)
                            nc.sync.dma_start(out_f[t][:, bh * 1024:(bh + 1) * 1024],
                                              fo[:, bh * 1024:(bh + 1) * 1024])
    nc.compile()
    return nc


# ---------------------------------------------------------------- entry point

def _run_hw(weight, bias, residual_scale, x, iters):
    from concourse.bass_utils import run_bass_kernel_spmd
    zero_bias = not np.any(bias)
    unit_rs = bool(np.all(residual_scale == 1.0))
    key = (iters, zero_bias, unit_rs)
    if key not in _CACHE:
        _CACHE[key] = _build(iters, zero_bias, unit_rs)
    nc = _CACHE[key]
    in_maps = _prep_inputs(weight, bias, residual_scale, x)
    res = run_bass_kernel_spmd(nc, in_maps, core_ids=list(range(N_CORES)))
    return _unpack_outputs(res.results)


def _np_blockify_param(p):
    lead = p.shape[:-3]
    y = p.reshape(*lead, 16, 8, 16, 8, 16, 8)
    n = len(lead)
    perm = tuple(range(n)) + (n, n + 2, n + 4, n + 1, n + 3, n + 5)
    return np.transpose(y, perm)


def _run_numpy(weight, bias, residual_scale, x, it):
    # reference fallback (pure numpy), used only if the HW path fails
    w = _np_blockify_param(weight)
    b = _np_blockify_param(bias)
    rs = _np_blockify_param(residual_scale)
    xp = np.pad(x, ((0, 0), (1, 1), (1, 1), (1, 1)))
    sw = np.lib.stride_tricks.sliding_window_view(xp, (10, 10, 10), axis=(1, 2, 3))
    acts = np.ascontiguousarray(sw[:, ::8, ::8, ::8])
    inter = slice(1, 9)
    for _ in range(it):
        mu = acts.mean(axis=(-3, -2, -1), keepdims=True)
        var = acts.var(axis=(-3, -2, -1), keepdims=True)
        normed = (acts - mu) / np.sqrt(var + EPS)
        acc = np.broadcast_to(b[None], (4,) + b.shape).copy()
        o = 0
        for di in range(3):
            for dj in range(3):
                for dk in range(3):
                    acc += w[o][None] * normed[..., di:di + 8, dj:dj + 8, dk:dk + 8]
                    o += 1
        acts[..., inter, inter, inter] += rs[None] * (acc / (1.0 + np.exp(-acc)))
    out = acts[..., inter, inter, inter]
    return out.transpose(0, 1, 4, 2, 5, 3, 6).reshape(4, 128, 128, 128).astype(np.float32)


def kernel(weight, bias, residual_scale, x, inner_iterations, block_size):
    weight = np.asarray(weight, np.float32)
    bias = np.asarray(bias, np.float32)
    residual_scale = np.asarray(residual_scale, np.float32)
    x = np.asarray(x, np.float32)
    it = int(inner_iterations)
    assert int(block_size) == BS and x.shape == (4, 128, 128, 128)
    try:
        return _run_hw(weight, bias, residual_scale, x, it)
    except Exception:
        import traceback
        traceback.print_exc()
        return _run_numpy(weight, bias, residual_scale, x, it)
